# revision 15
# baseline (speedup 1.0000x reference)
"""Bayesian 2-layer LSTM (S=256, B=64, H=1024) on 8 trn2 NeuronCores.

Strategy:
  - Host: sample weights w = mu + eps*softplus(rho) (threefry eps must match
    jax, so it is generated with jax on CPU), compute the KL scalar, and
    pre-layout weights/states for the device.
  - Device (8 cores, SPMD): both layers tensor-parallel over the H dimension
    (each core owns a 128-wide h-slice per layer -> its 4x128 gate rows).
    Per step: gates^T = W^T @ [x_t; h_{t-1}]^T accumulated in PSUM over 16
    K-chunks (weights stationary, fp16), gate nonlinearities on ACT with
    per-partition bias, c/h update on DVE, then an 8-core AllGather of the
    new h chunk. The two layers are independent and interleave, hiding each
    other's collective latency.
"""
import os
import sys
import numpy as np

for _p in ("/opt/trn_rl_repo",):
    if _p not in sys.path:
        sys.path.insert(0, _p)

S, B, H, L = 256, 64, 1024, 2
NCORES = 8
KK = 16          # contraction chunks (8 x-chunks + 8 h-chunks)
MCH = 4          # gate row chunks per core (i, f, o, j)
FORGET_BIAS = 1.0

_CACHE = {}


def _build(s_steps):
    import concourse.bacc as bacc
    import concourse.mybir as mybir
    from concourse import tile

    f32 = mybir.dt.float32
    f16 = mybir.dt.float16
    AF = mybir.ActivationFunctionType
    RG = [list(range(NCORES))]

    nc = bacc.Bacc("TRN2", target_bir_lowering=False, debug=False,
                   num_devices=NCORES)

    w_d = nc.dram_tensor("w", [L, 128, KK * MCH * 128], f16, kind="ExternalInput")
    bv_d = nc.dram_tensor("bv", [L, 1, MCH * 128], f16, kind="ExternalInput")
    ones_d = nc.inline_tensor(np.ones((1, B), np.float16), name="ones_row")
    xt_d = nc.dram_tensor("xt", [s_steps, H, B], f16, kind="ExternalInput")
    h0t_d = nc.dram_tensor("h0t", [L, 128, 8, B], f16, kind="ExternalInput")
    c0s_d = nc.dram_tensor("c0s", [L, 128, B], f32, kind="ExternalInput")

    oseq_d = nc.dram_tensor("out_seq", [s_steps, 128, B], f16, kind="ExternalOutput")
    hfin_d = nc.dram_tensor("h_fin", [L, 128, B], f16, kind="ExternalOutput")
    cfin_d = nc.dram_tensor("c_fin", [L, 128, B], f32, kind="ExternalOutput")

    with tile.TileContext(nc) as tc:
        with (
            tc.tile_pool(name="const", bufs=1) as constp,
            tc.tile_pool(name="sb", bufs=4) as sb,
            tc.tile_pool(name="dram", bufs=3, space="DRAM") as dp,
            tc.tile_pool(name="ps", bufs=3, space="PSUM") as ps,
        ):
            w_s = []
            bv_s = []
            ones_s = constp.tile([1, B], f16, name="ones_s")
            nc.sync.dma_start(ones_s[:], ones_d.ap())
            for l in range(L):
                wt = constp.tile([128, KK * MCH * 128], f16, name=f"w_s{l}")
                nc.sync.dma_start(wt[:], w_d.ap()[l])
                w_s.append(wt)
                bt = constp.tile([1, MCH * 128], f16, name=f"bv_s{l}")
                nc.sync.dma_start(bt[:], bv_d.ap()[l])
                bv_s.append(bt)

            # persistent per-layer state tiles: T2 = [tanh_j | c]
            T2 = []
            for l in range(L):
                t2 = constp.tile([128, 128], f32, name=f"T2_{l}")
                nc.sync.dma_start(t2[:, 64:128], c0s_d.ap()[l])
                T2.append(t2)

            hT = [None, None]
            for l in range(L):
                ht = sb.tile([128, 8, B], f16, tag=f"hTA{l}", name=f"ht0_{l}")
                nc.sync.dma_start(ht[:], h0t_d.ap()[l])
                hT[l] = (ht[:, 0:4], ht[:, 4:8])

            for t in range(s_steps):
                xt_t = sb.tile([128, 8, B], f16, tag="xt")
                nc.gpsimd.dma_start(
                    xt_t[:], xt_d.ap()[t].rearrange("(c p) b -> p c b", p=128))

                # Phase-interleaved emission: both layers' AG-independent work
                # is enqueued before either layer's AG-dependent work, so a
                # stalled h-matmul of one layer never head-of-line-blocks the
                # other layer's x-matmuls on the engine FIFOs.
                pgs = []
                for l in range(L):
                    pg = ps.tile([128, MCH * 64], f32, tag=f"g{l}",
                                 name=f"pg{l}")
                    pgs.append(pg)
                    wl = w_s[l]
                    first = True
                    for kk in range(8):
                        for m in range(MCH):
                            toff = (kk * MCH + m) * 128
                            nc.tensor.matmul(
                                pg[:, 64 * m:64 * m + 64],
                                wl[:, toff:toff + 128],
                                xt_t[:, kk, :],
                                start=first, stop=False,
                            )
                            first = False
                    for m in range(MCH):
                        nc.tensor.matmul(
                            pg[:, 64 * m:64 * m + 64],
                            bv_s[l][:, 128 * m:128 * m + 128],
                            ones_s[:],
                            start=False, stop=False,
                        )

                for l in range(L):
                    pg, wl = pgs[l], w_s[l]
                    htA, htB = hT[l]
                    for src, half in ((htA, 0), (htB, 1)):
                        for kk in range(4):
                            for m in range(MCH):
                                toff = ((8 + half * 4 + kk) * MCH + m) * 128
                                nc.tensor.matmul(
                                    pg[:, 64 * m:64 * m + 64],
                                    wl[:, toff:toff + 128],
                                    src[:, kk, :],
                                    start=False,
                                    stop=(half == 1 and kk == 3 and m == MCH - 1),
                                )

                    actS = sb.tile([128, 192], f32, tag=f"actS{l}",
                                   name=f"actS{l}")
                    # m-chunk order: 0=i, 1=f, 2=o, 3=j
                    nc.scalar.activation(actS[:], pg[:, 0:192], AF.Sigmoid)
                    nc.scalar.activation(T2[l][:, 0:64], pg[:, 192:256], AF.Tanh)
                    P12 = sb.tile([128, 128], f32, tag=f"P12{l}",
                                  name=f"P12{l}")
                    nc.vector.tensor_mul(P12[:], actS[:, 0:128], T2[l][:])
                    nc.vector.tensor_add(T2[l][:, 64:128], P12[:, 0:64],
                                         P12[:, 64:128])
                    tcn = sb.tile([128, 64], f32, tag=f"tcn{l}", name=f"tcn{l}")
                    nc.scalar.activation(tcn[:], T2[l][:, 64:128], AF.Tanh)
                    hnew = sb.tile([128, 64], f16, tag=f"hnew{l}",
                                   name=f"hnew{l}")
                    nc.vector.tensor_mul(hnew[:], tcn[:], actS[:, 128:192])

                    if l == L - 1:
                        nc.gpsimd.dma_start(oseq_d.ap()[t], hnew[:])

                    if t == s_steps - 1:
                        nc.sync.dma_start(hfin_d.ap()[l], hnew[:])
                        nc.sync.dma_start(cfin_d.ap()[l], T2[l][:, 64:128])
                    else:
                        agin = dp.tile([128, B], f16, tag=f"agin{l}",
                                       name=f"agin{l}")
                        agout = dp.tile([NCORES * 128, B], f16, tag=f"agout{l}",
                                        addr_space="Shared", name=f"agout{l}")
                        nc.scalar.dma_start(agin[:], hnew[:])
                        nc.gpsimd.collective_compute(
                            "AllGather", mybir.AluOpType.bypass,
                            replica_groups=RG,
                            ins=[agin.opt()], outs=[agout.opt()],
                        )
                        ht = sb.tile([128, 8, B], f16, tag=f"hTA{l}",
                                     name=f"ht{l}")
                        nc.sync.dma_start(
                            ht[:], agout.rearrange("(c p) b -> p c b", p=128))
                        hT[l] = (ht[:, 0:4], ht[:, 4:8])

    nc.compile()
    return nc


def _host_prepare(x, h0, c0, mu, rho, bias):
    """Sample weights, compute KL, build per-core device arrays."""
    import jax
    import jax.numpy as jnp
    cpu = jax.devices("cpu")[0]

    f16 = np.float16
    n_steps = x.shape[0]
    kl = np.float64(0.0)
    LOG2PI = np.log(2.0 * np.pi)
    S1, S2, PI_MIX = np.exp(-1.0), np.exp(-7.0), 0.25

    w_cores = [[None] * L for _ in range(NCORES)]
    bv_cores = [[None] * L for _ in range(NCORES)]
    for l in range(L):
        rho_l = rho[l].astype(np.float64)
        sigma = np.logaddexp(0.0, rho_l) + 1e-5
        with jax.default_device(cpu):
            eps = np.asarray(jax.random.normal(
                jax.random.fold_in(jax.random.key(1), l),
                mu[l].shape, dtype=jnp.float32)).astype(np.float64)
        w = mu[l].astype(np.float64) + eps * sigma
        # KL in float64 (reduction over 8.4M elements)
        log_q = np.sum(-0.5 * LOG2PI - np.log(sigma) - 0.5 * eps * eps)
        lp1 = -0.5 * LOG2PI - np.log(S1) - 0.5 * (w / S1) ** 2 + np.log(PI_MIX)
        lp2 = -0.5 * LOG2PI - np.log(S2) - 0.5 * (w / S2) ** 2 + np.log(1.0 - PI_MIX)
        log_p = np.sum(np.logaddexp(lp1, lp2))
        kl += log_q - log_p

        wf = w.astype(np.float32)
        for k in range(NCORES):
            sl = slice(128 * k, 128 * k + 128)
            # m-chunk order [i, f, o, j]; reference gate order is i, j, f, o
            cols = np.concatenate([
                np.arange(0, H)[sl],            # i
                np.arange(2 * H, 3 * H)[sl],    # f
                np.arange(3 * H, 4 * H)[sl],    # o
                np.arange(H, 2 * H)[sl],        # j
            ])
            wc = wf[:, cols].astype(f16)                      # [2048, 512]
            wt = wc.reshape(KK, 128, MCH, 128).transpose(1, 0, 2, 3)
            w_cores[k][l] = np.ascontiguousarray(
                wt.reshape(128, KK * MCH * 128))
            bp = bias[l][cols].astype(np.float32).reshape(MCH, 128).copy()
            bp[1] += FORGET_BIAS
            bv_cores[k][l] = np.ascontiguousarray(
                bp.reshape(1, MCH * 128).astype(f16))         # [1, MCH*128]

    xt = np.ascontiguousarray(
        x[:n_steps].transpose(0, 2, 1).astype(f16))           # [S, H, B]
    h0t = np.ascontiguousarray(
        h0.transpose(0, 2, 1).reshape(L, 8, 128, B).transpose(0, 2, 1, 3)
        .astype(f16))                                         # [L, 128, 8, B]

    in_maps = []
    for k in range(NCORES):
        c0s = np.ascontiguousarray(
            c0[:, :, 128 * k:128 * k + 128].transpose(0, 2, 1)
            .astype(np.float32))                              # [L, 128, B]
        in_maps.append({
            "w": np.ascontiguousarray(np.stack(w_cores[k])),
            "bv": np.ascontiguousarray(np.stack(bv_cores[k])),
            "xt": xt,
            "h0t": h0t,
            "c0s": c0s,
        })
    return in_maps, np.float32(kl)


def kernel(x, h0, c0, mu, rho, bias):
    from concourse import bass_utils

    x = np.asarray(x, np.float32)
    h0 = np.asarray(h0, np.float32)
    c0 = np.asarray(c0, np.float32)
    mu = np.asarray(mu, np.float32)
    rho = np.asarray(rho, np.float32)
    bias = np.asarray(bias, np.float32)

    n_steps = x.shape[0]
    in_maps, kl = _host_prepare(x, h0, c0, mu, rho, bias)

    if n_steps not in _CACHE:
        _CACHE[n_steps] = _build(n_steps)
    nc = _CACHE[n_steps]

    trace = bool(int(os.environ.get("BASS_LSTM_TRACE", "0")))
    res = bass_utils.run_bass_kernel_spmd(
        nc, in_maps, core_ids=list(range(NCORES)), trace=trace)
    kernel.last_results = res

    out = np.empty((n_steps, B, H), np.float32)
    h_fin = np.empty((L, B, H), np.float32)
    c_fin = np.empty((L, B, H), np.float32)
    for k in range(NCORES):
        r = res.results[k]
        sl = slice(128 * k, 128 * k + 128)
        out[:, :, sl] = r["out_seq"].astype(np.float32).transpose(0, 2, 1)
        h_fin[:, :, sl] = r["h_fin"].astype(np.float32).transpose(0, 2, 1)
        c_fin[:, :, sl] = r["c_fin"].astype(np.float32).transpose(0, 2, 1)
    return out, h_fin, c_fin, kl


# revision 21
# speedup vs baseline: 23.6614x; 23.6614x over previous
"""Bayesian 2-layer LSTM (S=256, B=64, H=1024) on 8 trn2 NeuronCores.

Strategy:
  - Host: sample weights w = mu + eps*softplus(rho) (threefry eps must match
    jax, so it is generated with jax on CPU), compute the KL scalar, and
    pre-layout weights/states for the device.
  - Device (8 cores, SPMD): both layers tensor-parallel over the H dimension
    (each core owns a 128-wide h-slice per layer -> its 4x128 gate rows).
    Per step: gates^T = W^T @ [x_t; h_{t-1}]^T accumulated in PSUM over 16
    K-chunks (weights stationary, fp16), gate nonlinearities on ACT with
    per-partition bias, c/h update on DVE, then an 8-core AllGather of the
    new h chunk. The two layers are independent and interleave, hiding each
    other's collective latency.
"""
import os
import sys
import numpy as np

for _p in ("/opt/trn_rl_repo",):
    if _p not in sys.path:
        sys.path.insert(0, _p)

S, B, H, L = 256, 64, 1024, 2
NCORES = 8
KK = 16          # contraction chunks (8 x-chunks + 8 h-chunks)
MCH = 4          # gate row chunks per core (i, f, o, j)
FORGET_BIAS = 1.0

_CACHE = {}


def _build(s_steps):
    import concourse.bacc as bacc
    import concourse.mybir as mybir
    from concourse import tile

    f32 = mybir.dt.float32
    f16 = mybir.dt.float16
    AF = mybir.ActivationFunctionType
    RG = [list(range(NCORES))]

    nc = bacc.Bacc("TRN2", target_bir_lowering=False, debug=False,
                   num_devices=NCORES)

    w_d = nc.dram_tensor("w", [L, 128, KK * MCH * 128], f16, kind="ExternalInput")
    bv_d = nc.dram_tensor("bv", [L, 1, MCH * 128], f16, kind="ExternalInput")
    ones_d = nc.inline_tensor(np.ones((1, B), np.float16), name="ones_row")
    xt_d = nc.dram_tensor("xt", [s_steps, H, B], f16, kind="ExternalInput")
    h0t_d = nc.dram_tensor("h0t", [L, 128, 8, B], f16, kind="ExternalInput")
    c0s_d = nc.dram_tensor("c0s", [L, 128, B], f32, kind="ExternalInput")

    oseq_d = nc.dram_tensor("out_seq", [s_steps, 128, B], f16, kind="ExternalOutput")
    hfin_d = nc.dram_tensor("h_fin", [L, 128, B], f16, kind="ExternalOutput")
    cfin_d = nc.dram_tensor("c_fin", [L, 128, B], f32, kind="ExternalOutput")

    with tile.TileContext(nc) as tc:
        with (
            tc.tile_pool(name="const", bufs=1) as constp,
            tc.tile_pool(name="sb", bufs=4) as sb,
            tc.tile_pool(name="dram", bufs=3, space="DRAM") as dp,
            tc.tile_pool(name="ps", bufs=3, space="PSUM") as ps,
        ):
            w_s = []
            bv_s = []
            ones_s = constp.tile([1, B], f16, name="ones_s")
            nc.sync.dma_start(ones_s[:], ones_d.ap())
            for l in range(L):
                wt = constp.tile([128, KK * MCH * 128], f16, name=f"w_s{l}")
                nc.sync.dma_start(wt[:], w_d.ap()[l])
                w_s.append(wt)
                bt = constp.tile([1, MCH * 128], f16, name=f"bv_s{l}")
                nc.sync.dma_start(bt[:], bv_d.ap()[l])
                bv_s.append(bt)

            # persistent per-layer state tiles: T2 = [tanh_j | c]
            T2 = []
            for l in range(L):
                t2 = constp.tile([128, 128], f32, name=f"T2_{l}")
                nc.sync.dma_start(t2[:, 64:128], c0s_d.ap()[l])
                T2.append(t2)

            hT = [None, None]
            for l in range(L):
                ht = sb.tile([128, 8, B], f16, tag=f"hTA{l}", name=f"ht0_{l}")
                nc.sync.dma_start(ht[:], h0t_d.ap()[l])
                hT[l] = (ht[:, 0:4], ht[:, 4:8])

            for t in range(s_steps):
                xt_t = sb.tile([128, 8, B], f16, tag="xt")
                nc.sync.dma_start(
                    xt_t[:], xt_d.ap()[t].rearrange("(c p) b -> p c b", p=128))

                # Phase-interleaved emission: both layers' AG-independent work
                # is enqueued before either layer's AG-dependent work, so a
                # stalled h-matmul of one layer never head-of-line-blocks the
                # other layer's x-matmuls on the engine FIFOs.
                pgs = []
                for l in range(L):
                    pg = ps.tile([128, MCH * 64], f32, tag=f"g{l}",
                                 name=f"pg{l}")
                    pgs.append(pg)
                    wl = w_s[l]
                    first = True
                    for kk in range(8):
                        for m in range(MCH):
                            toff = (kk * MCH + m) * 128
                            nc.tensor.matmul(
                                pg[:, 64 * m:64 * m + 64],
                                wl[:, toff:toff + 128],
                                xt_t[:, kk, :],
                                start=first, stop=False,
                            )
                            first = False
                    for m in range(MCH):
                        nc.tensor.matmul(
                            pg[:, 64 * m:64 * m + 64],
                            bv_s[l][:, 128 * m:128 * m + 128],
                            ones_s[:],
                            start=False, stop=False,
                        )

                for l in range(L):
                    pg, wl = pgs[l], w_s[l]
                    htA, htB = hT[l]
                    for src, half in ((htA, 0), (htB, 1)):
                        for kk in range(4):
                            for m in range(MCH):
                                toff = ((8 + half * 4 + kk) * MCH + m) * 128
                                nc.tensor.matmul(
                                    pg[:, 64 * m:64 * m + 64],
                                    wl[:, toff:toff + 128],
                                    src[:, kk, :],
                                    start=False,
                                    stop=(half == 1 and kk == 3 and m == MCH - 1),
                                )

                    actS = sb.tile([128, 192], f32, tag=f"actS{l}",
                                   name=f"actS{l}")
                    # m-chunk order: 0=i, 1=f, 2=o, 3=j
                    nc.scalar.activation(actS[:], pg[:, 0:192], AF.Sigmoid)
                    nc.scalar.activation(T2[l][:, 0:64], pg[:, 192:256], AF.Tanh)
                    P12 = sb.tile([128, 128], f32, tag=f"P12{l}",
                                  name=f"P12{l}")
                    nc.vector.tensor_mul(P12[:], actS[:, 0:128], T2[l][:])
                    nc.vector.tensor_add(T2[l][:, 64:128], P12[:, 0:64],
                                         P12[:, 64:128])
                    tcn = sb.tile([128, 64], f32, tag=f"tcn{l}", name=f"tcn{l}")
                    nc.scalar.activation(tcn[:], T2[l][:, 64:128], AF.Tanh)
                    hnew = sb.tile([128, 64], f16, tag=f"hnew{l}",
                                   name=f"hnew{l}")
                    nc.vector.tensor_mul(hnew[:], tcn[:], actS[:, 128:192])

                    if l == L - 1:
                        nc.sync.dma_start(oseq_d.ap()[t], hnew[:])

                    if t == s_steps - 1:
                        nc.sync.dma_start(hfin_d.ap()[l], hnew[:])
                        nc.sync.dma_start(cfin_d.ap()[l], T2[l][:, 64:128])
                    else:
                        if l == 0:
                            agin = dp.tile([128, B], f16, tag=f"agin{l}",
                                           name=f"agin{l}")
                            agout = dp.tile([NCORES * 128, B], f16,
                                            tag=f"agout{l}",
                                            addr_space="Shared",
                                            name=f"agout{l}")
                            nc.scalar.dma_start(agin[:], hnew[:])
                            nc.gpsimd.collective_compute(
                                "AllGather", mybir.AluOpType.bypass,
                                replica_groups=RG,
                                ins=[agin.opt()], outs=[agout.opt()],
                            )
                        else:
                            agin = dp.tile([NCORES * 128, B], f16,
                                           tag=f"agin{l}", name=f"agin{l}")
                            agout = dp.tile([NCORES * 128, B], f16,
                                            tag=f"agout{l}",
                                            name=f"agout{l}")
                            nc.scalar.dma_start(
                                agin.rearrange("(c p) b -> p c b", p=128),
                                hnew[:].rearrange("p (o b) -> p o b", o=1).to_broadcast([128, NCORES, B]))
                            nc.gpsimd.collective_compute(
                                "AllToAll", mybir.AluOpType.bypass,
                                replica_groups=RG,
                                ins=[agin.opt()], outs=[agout.opt()],
                            )
                        ht = sb.tile([128, 8, B], f16, tag=f"hTA{l}",
                                     name=f"ht{l}")
                        nc.sync.dma_start(
                            ht[:], agout.rearrange("(c p) b -> p c b", p=128))
                        hT[l] = (ht[:, 0:4], ht[:, 4:8])

    nc.compile()
    return nc


def _host_prepare(x, h0, c0, mu, rho, bias):
    """Sample weights, compute KL, build per-core device arrays."""
    import jax
    import jax.numpy as jnp
    cpu = jax.devices("cpu")[0]

    f16 = np.float16
    n_steps = x.shape[0]
    kl = np.float64(0.0)
    LOG2PI = np.log(2.0 * np.pi)
    S1, S2, PI_MIX = np.exp(-1.0), np.exp(-7.0), 0.25

    w_cores = [[None] * L for _ in range(NCORES)]
    bv_cores = [[None] * L for _ in range(NCORES)]
    for l in range(L):
        rho_l = rho[l].astype(np.float64)
        sigma = np.logaddexp(0.0, rho_l) + 1e-5
        with jax.default_device(cpu):
            eps = np.asarray(jax.random.normal(
                jax.random.fold_in(jax.random.key(1), l),
                mu[l].shape, dtype=jnp.float32)).astype(np.float64)
        w = mu[l].astype(np.float64) + eps * sigma
        # KL in float64 (reduction over 8.4M elements)
        log_q = np.sum(-0.5 * LOG2PI - np.log(sigma) - 0.5 * eps * eps)
        lp1 = -0.5 * LOG2PI - np.log(S1) - 0.5 * (w / S1) ** 2 + np.log(PI_MIX)
        lp2 = -0.5 * LOG2PI - np.log(S2) - 0.5 * (w / S2) ** 2 + np.log(1.0 - PI_MIX)
        log_p = np.sum(np.logaddexp(lp1, lp2))
        kl += log_q - log_p

        wf = w.astype(np.float32)
        for k in range(NCORES):
            sl = slice(128 * k, 128 * k + 128)
            # m-chunk order [i, f, o, j]; reference gate order is i, j, f, o
            cols = np.concatenate([
                np.arange(0, H)[sl],            # i
                np.arange(2 * H, 3 * H)[sl],    # f
                np.arange(3 * H, 4 * H)[sl],    # o
                np.arange(H, 2 * H)[sl],        # j
            ])
            wc = wf[:, cols].astype(f16)                      # [2048, 512]
            wt = wc.reshape(KK, 128, MCH, 128).transpose(1, 0, 2, 3)
            w_cores[k][l] = np.ascontiguousarray(
                wt.reshape(128, KK * MCH * 128))
            bp = bias[l][cols].astype(np.float32).reshape(MCH, 128).copy()
            bp[1] += FORGET_BIAS
            bv_cores[k][l] = np.ascontiguousarray(
                bp.reshape(1, MCH * 128).astype(f16))         # [1, MCH*128]

    xt = np.ascontiguousarray(
        x[:n_steps].transpose(0, 2, 1).astype(f16))           # [S, H, B]
    h0t = np.ascontiguousarray(
        h0.transpose(0, 2, 1).reshape(L, 8, 128, B).transpose(0, 2, 1, 3)
        .astype(f16))                                         # [L, 128, 8, B]

    in_maps = []
    for k in range(NCORES):
        c0s = np.ascontiguousarray(
            c0[:, :, 128 * k:128 * k + 128].transpose(0, 2, 1)
            .astype(np.float32))                              # [L, 128, B]
        in_maps.append({
            "w": np.ascontiguousarray(np.stack(w_cores[k])),
            "bv": np.ascontiguousarray(np.stack(bv_cores[k])),
            "xt": xt,
            "h0t": h0t,
            "c0s": c0s,
        })
    return in_maps, np.float32(kl)


def kernel(x, h0, c0, mu, rho, bias):
    from concourse import bass_utils

    x = np.asarray(x, np.float32)
    h0 = np.asarray(h0, np.float32)
    c0 = np.asarray(c0, np.float32)
    mu = np.asarray(mu, np.float32)
    rho = np.asarray(rho, np.float32)
    bias = np.asarray(bias, np.float32)

    n_steps = x.shape[0]
    in_maps, kl = _host_prepare(x, h0, c0, mu, rho, bias)

    if n_steps not in _CACHE:
        _CACHE[n_steps] = _build(n_steps)
    nc = _CACHE[n_steps]

    trace = bool(int(os.environ.get("BASS_LSTM_TRACE", "0")))
    res = bass_utils.run_bass_kernel_spmd(
        nc, in_maps, core_ids=list(range(NCORES)), trace=trace)
    kernel.last_results = res

    out = np.empty((n_steps, B, H), np.float32)
    h_fin = np.empty((L, B, H), np.float32)
    c_fin = np.empty((L, B, H), np.float32)
    for k in range(NCORES):
        r = res.results[k]
        sl = slice(128 * k, 128 * k + 128)
        out[:, :, sl] = r["out_seq"].astype(np.float32).transpose(0, 2, 1)
        h_fin[:, :, sl] = r["h_fin"].astype(np.float32).transpose(0, 2, 1)
        c_fin[:, :, sl] = r["c_fin"].astype(np.float32).transpose(0, 2, 1)
    return out, h_fin, c_fin, kl
